# revision 1
# baseline (speedup 1.0000x reference)
"""Trainium2 Bass kernel for the BreakthroughSNN encoder problem.

Computation (per (b, t, s, d) element):
    w = softmax(enc_weights)  (4 scalars, host)
    rates   = clip(sigmoid(emb)*0.9 + 0.05 + 0.1*noise, 0, 1)         [b,s,d]
    rate    = 1[rate_rand < rates]                                    [b,t,s,d]
    st      = floor(sigmoid(emb) * (T-1))                             [b,s,d]
    temporal= 1[st == t]                                              [b,t,s,d]
    presp   = emb @ pop_W + pop_b ; prates = sigmoid(presp)           [b,s,d,n]
    pop     = mean_n 1[pop_rand < prates]                             [b,t,s,d]
    waves   = sin(freq_d * t_k + sigmoid(emb)*2pi)                    [b,t,s,d]
    phase   = 1[waves > 0.5]                                          [b,t,s,d]
    out     = w0*rate + w1*temporal + w2*pop + w3*phase

Sharding: the (b, s) token axis (4*256 = 1024 tokens) is split evenly
across 8 NeuronCores (128 tokens/core, = SBUF partition dim).  pop_W is
replicated.  Host pre-transposes rate_rand/pop_rand into per-core
[t][token][feature] slabs (pop features n-major so the N-reduction is a
contiguous halving tree), launches one SPMD Bass program on cores 0-7,
and re-assembles the full [B,T,S,D] output.
"""

import os
import sys

for _p in ("/opt/trn_rl_repo", os.path.expanduser("~/.axon_site/_ro/trn_rl_repo")):
    if os.path.isdir(_p) and _p not in sys.path:
        sys.path.insert(0, _p)

import numpy as np

import concourse.bacc as bacc
import concourse.mybir as mybir
import concourse.tile as tile
from concourse.bass_utils import run_bass_kernel_spmd

Alu = mybir.AluOpType
Act = mybir.ActivationFunctionType
F32 = mybir.dt.float32
BF16 = mybir.dt.bfloat16

TWO_PI = 2.0 * np.pi

B, T, S, D, N = 4, 16, 256, 512, 8
NCORES = 8
NTOK = B * S                 # 1024 tokens
TOK = NTOK // NCORES         # 128 tokens per core (= partition dim)
DN = D * N                   # 4096
HF = DN // 2                 # 2048


def _build_program(w0, w1, w2, w3, has_bias):
    """Build the single-core Bass/Tile program (run SPMD on 8 cores)."""
    from contextlib import ExitStack

    nk = D // 128 + (1 if has_bias else 0)   # K-chunks of the pop matmul
    kdim = nk * 128
    uniform = abs(w1 - w0) < 1e-12 and abs(w3 - w0) < 1e-12
    c_pop = w2 / (N * w0) if uniform else w2 / N

    nc = bacc.Bacc("TRN2", target_bir_lowering=False, debug=False,
                   num_devices=NCORES)

    emb = nc.dram_tensor("emb", [TOK, D], F32, kind="ExternalInput")
    embT = nc.dram_tensor("embT", [kdim, TOK], F32, kind="ExternalInput")
    noise = nc.dram_tensor("noise", [TOK, D], F32, kind="ExternalInput")
    rr = nc.dram_tensor("rr", [T, TOK, D], F32, kind="ExternalInput")
    pr = nc.dram_tensor("pr", [T, TOK, DN], F32, kind="ExternalInput")
    Wd = nc.dram_tensor("W", [kdim, DN], F32, kind="ExternalInput")
    tfd = nc.dram_tensor("tf", [3, T * D], F32, kind="ExternalInput")
    identd = nc.dram_tensor("ident", [128, 128], F32, kind="ExternalInput")
    outd = nc.dram_tensor("out", [T, TOK, D], F32, kind="ExternalOutput")

    with tile.TileContext(nc) as tc, ExitStack() as ctx:
        const = ctx.enter_context(tc.tile_pool(name="const", bufs=1))
        tfp = ctx.enter_context(tc.tile_pool(name="tfp", bufs=1))
        wp = ctx.enter_context(tc.tile_pool(name="wp", bufs=2))
        psum = ctx.enter_context(tc.tile_pool(name="psum", bufs=2, space="PSUM"))
        lp = ctx.enter_context(tc.tile_pool(name="lp", bufs=2))

        # ---- constants / one-time loads ----
        ident = const.tile([128, 128], F32)
        nc.sync.dma_start(ident[:], identd[:])
        ones_row = const.tile([1, 128], F32)
        nc.vector.memset(ones_row[:], 1.0)
        emb_sb = const.tile([TOK, D], F32)
        nc.sync.dma_start(emb_sb[:], emb[:])
        noise_sb = const.tile([TOK, D], F32)
        nc.sync.dma_start(noise_sb[:], noise[:])
        lhsT = []
        for k in range(nk):
            lt = const.tile([128, TOK], F32, tag=f"lhsT{k}")
            nc.sync.dma_start(lt[:], embT[k * 128:(k + 1) * 128, :])
            lhsT.append(lt)

        # ---- per-token precompute ----
        sig = const.tile([TOK, D], F32)
        nc.scalar.activation(sig[:], emb_sb[:], Act.Sigmoid)

        rates = const.tile([TOK, D], F32)
        tmp = const.tile([TOK, D], F32)
        # tmp = sig*0.9 + 0.05 ; tmp += 0.1*noise ; rates = clip(tmp,0,1)
        nc.vector.tensor_scalar(tmp[:], sig[:], 0.9, 0.05, Alu.mult, Alu.add)
        nc.vector.scalar_tensor_tensor(tmp[:], noise_sb[:], 0.1, tmp[:],
                                       Alu.mult, Alu.add)
        nc.vector.tensor_scalar(rates[:], tmp[:], 0.0, 1.0, Alu.max, Alu.min)

        # st = floor(sig*(T-1)):  rnd = RNE(x) via +-2^23, st = rnd - 1[rnd > x]
        st = const.tile([TOK, D], F32)
        x15 = const.tile([TOK, D], F32)
        nc.vector.tensor_scalar(x15[:], sig[:], float(T - 1), None, Alu.mult)
        rnd = const.tile([TOK, D], F32)
        nc.vector.tensor_scalar(rnd[:], x15[:], 8388608.0, 8388608.0,
                                Alu.add, Alu.subtract)
        gtt = const.tile([TOK, D], F32)
        nc.vector.tensor_tensor(gtt[:], rnd[:], x15[:], Alu.is_gt)
        nc.vector.tensor_tensor(st[:], rnd[:], gtt[:], Alu.subtract)

        phases = const.tile([TOK, D], F32)
        nc.vector.tensor_scalar(phases[:], sig[:], TWO_PI, None, Alu.mult)

        # ---- pop linear: presp = emb @ W (+ b folded in via extra K rows) ----
        prt = const.tile([TOK, DN], F32)         # pop rates, n-major columns
        for h in range(2):
            ps = psum.tile([128, HF], F32, tag="pp")
            for k in range(nk):
                wt = wp.tile([128, HF], F32, tag="w")
                nc.sync.dma_start(wt[:], Wd[k * 128:(k + 1) * 128,
                                            h * HF:(h + 1) * HF])
                for j in range(HF // 512):
                    sl = slice(j * 512, (j + 1) * 512)
                    nc.tensor.matmul(ps[:, sl], lhsT[k][:], wt[:, sl],
                                     start=(k == 0), stop=(k == nk - 1))
            nc.scalar.activation(prt[:, h * HF:(h + 1) * HF], ps[:], Act.Sigmoid)

        # ---- waves = sin(phases + t_k * freq), computed per 2-t chunk and
        # interleaved with the t-loop so PE/ACT overlap the DVE stream ----
        # ACT Sin is only valid on [-pi, pi]; the argument reaches ~69.
        # PE accumulates  arg = (((phases + tf) - k0*c_hi) - k0*c_lo)
        # in this exact order (first add reproduces jax's f32 rounding at
        # full magnitude; the k0*c_hi subtract is Sterbenz-exact), giving
        # arg1 in (-pi, 3pi).  Fold to (-pi, pi]: arg -= 2pi*1[arg >= pi],
        # with the 0/1 indicator built on ACT (Relu then Sign) to keep the
        # DVE free; the fold subtract is exact for arg < pi.
        waves = const.tile([TOK, T * D], F32)
        CH = 1024                                # arg chunk width (2 t-steps)
        PI_F = float(np.float32(np.pi))
        neg_pi = const.tile([128, 1], F32)
        nc.vector.memset(neg_pi[:], -PI_F)

        def emit_waves_chunk(ch):
            tf_rows = []
            for r in range(3):
                trow = tfp.tile([1, CH], F32, name=f"tfr{r}", tag=f"tf{r}")
                nc.sync.dma_start(trow[:], tfd[r:r + 1, ch * CH:(ch + 1) * CH])
                tf_rows.append(trow)
            ps = psum.tile([128, CH], F32, name="ps_arg", tag="pp")
            for j in range(CH // 512):
                sl = slice(j * 512, (j + 1) * 512)
                nc.tensor.matmul(ps[:, sl], ident[:], phases[:],
                                 start=True, stop=False)
            for r in range(3):
                for j in range(CH // 512):
                    sl = slice(j * 512, (j + 1) * 512)
                    nc.tensor.matmul(ps[:, sl], ones_row[:], tf_rows[r][0:1, sl],
                                     start=False, stop=(r == 2))
            fold = tfp.tile([TOK, CH], F32, name="fold", tag="fold")
            nc.scalar.activation(fold[:], ps[:], Act.Relu, bias=neg_pi[:])
            nc.scalar.activation(fold[:], fold[:], Act.Sign)
            argf = tfp.tile([TOK, CH], F32, name="argf", tag="argf")
            nc.vector.scalar_tensor_tensor(argf[:], fold[:],
                                           -float(np.float32(TWO_PI)), ps[:],
                                           Alu.mult, Alu.add)
            nc.scalar.activation(waves[:, ch * CH:(ch + 1) * CH], argf[:],
                                 Act.Sin)

        # ---- t-loop ----
        for t in range(T):
            if t % 2 == 0:
                emit_waves_chunk(t // 2)
            pt = lp.tile([TOK, DN], F32, tag="pt")
            nc.sync.dma_start(pt[:], pr[t])
            rt = lp.tile([TOK, D], F32, tag="rt")
            nc.sync.dma_start(rt[:], rr[t])

            spk = lp.tile([TOK, DN], BF16, tag="spk")
            nc.vector.tensor_tensor(spk[:, 0:HF], pt[:, 0:HF], prt[:, 0:HF],
                                    Alu.is_lt)
            nc.vector.tensor_tensor(spk[:, HF:DN], pt[:, HF:DN], prt[:, HF:DN],
                                    Alu.is_lt)
            # halving tree over n (n-major layout -> contiguous adds)
            h1 = lp.tile([TOK, HF], BF16, tag="h1")
            nc.vector.tensor_tensor(h1[:], spk[:, 0:HF], spk[:, HF:DN], Alu.add)
            h2 = lp.tile([TOK, HF // 2], BF16, tag="h2")
            nc.vector.tensor_tensor(h2[:], h1[:, 0:HF // 2], h1[:, HF // 2:HF],
                                    Alu.add)
            pops = lp.tile([TOK, D], BF16, tag="pops")
            nc.vector.tensor_tensor(pops[:], h2[:, 0:D], h2[:, D:2 * D], Alu.add)

            # temporal one-hot via two inequalities (STT is_equal is a
            # masking select on HW, not a 0/1 compare):
            #   1[st==t] = 1[st > t-0.5] + 1[st < t+0.5] - 1
            # The -1 is folded into the final ACT bias.
            sA = lp.tile([TOK, D], F32, tag="sA")
            sB = lp.tile([TOK, D], F32, tag="sB")
            wv = waves[:, t * D:(t + 1) * D]
            nc.vector.tensor_tensor(sA[:], rt[:], rates[:], Alu.is_lt)
            if uniform:
                nc.vector.scalar_tensor_tensor(sB[:], st[:], t - 0.5, sA[:],
                                               Alu.is_gt, Alu.add)
                nc.vector.scalar_tensor_tensor(sA[:], st[:], t + 0.5, sB[:],
                                               Alu.is_lt, Alu.add)
                nc.vector.scalar_tensor_tensor(sB[:], wv, 0.5, sA[:],
                                               Alu.is_gt, Alu.add)
                nc.vector.scalar_tensor_tensor(sA[:], pops[:], c_pop, sB[:],
                                               Alu.mult, Alu.add)
                ot = lp.tile([TOK, D], F32, tag="ot")
                nc.scalar.activation(ot[:], sA[:], Act.Copy, bias=-w0, scale=w0)
            else:
                nc.vector.tensor_scalar(sA[:], sA[:], w0, None, Alu.mult)
                gA = lp.tile([TOK, D], F32, tag="gA")
                gB = lp.tile([TOK, D], F32, tag="gB")
                nc.vector.tensor_scalar(gA[:], st[:], t - 0.5, None, Alu.is_gt)
                nc.vector.tensor_scalar(gB[:], st[:], t + 0.5, None, Alu.is_lt)
                nc.vector.tensor_tensor(gA[:], gA[:], gB[:], Alu.mult)
                nc.vector.scalar_tensor_tensor(sB[:], gA[:], w1, sA[:],
                                               Alu.mult, Alu.add)
                nc.vector.tensor_scalar(gB[:], wv, 0.5, None, Alu.is_gt)
                nc.vector.scalar_tensor_tensor(sA[:], gB[:], w3, sB[:],
                                               Alu.mult, Alu.add)
                nc.vector.scalar_tensor_tensor(sB[:], pops[:], c_pop, sA[:],
                                               Alu.mult, Alu.add)
                ot = lp.tile([TOK, D], F32, tag="ot")
                nc.scalar.activation(ot[:], sB[:], Act.Copy, bias=0.0, scale=1.0)
            nc.sync.dma_start(outd[t], ot[:])

    nc.compile()
    return nc


def _prepare_inputs(embeddings, pop_W, pop_b, freq_bands, enc_weights,
                    rate_noise, rate_rand, pop_rand):
    """Host-side sharding + layout transforms -> per-core in_maps."""
    e = np.exp(enc_weights.astype(np.float64)
               - enc_weights.astype(np.float64).max())
    w = (e / e.sum()).astype(np.float32)
    w0, w1, w2, w3 = [float(x) for x in w]

    has_bias = bool(np.any(pop_b != 0))
    kdim = D + (128 if has_bias else 0)

    emb_f = np.ascontiguousarray(embeddings.reshape(NTOK, D))
    noise_f = np.ascontiguousarray(rate_noise.reshape(NTOK, D))
    # rate_rand [B,T,S,D] -> [BS, T, D]
    rr_f = np.ascontiguousarray(rate_rand.transpose(0, 2, 1, 3)
                                .reshape(NTOK, T, D))
    # pop_rand [B,T,S,D,N] -> [BS, T, N, D] (n-major feature axis)
    pr_f = np.ascontiguousarray(pop_rand.transpose(0, 2, 1, 4, 3)
                                .reshape(NTOK, T, DN))
    # pop_W columns reordered to n-major: W2[k, n*D+d] = pop_W[k, d*N+n]
    W2 = np.ascontiguousarray(pop_W.reshape(D, D, N).transpose(0, 2, 1)
                              .reshape(D, DN))
    if has_bias:
        b_nm = np.ascontiguousarray(pop_b.reshape(D, N).T.reshape(1, DN))
        W2 = np.vstack([W2, b_nm, np.zeros((127, DN), np.float32)])
    W2 = np.ascontiguousarray(W2.astype(np.float32))

    # match jnp.linspace bit-exactly (grader's reference runs jax-on-cpu)
    import jax
    import jax.numpy as jnp
    with jax.default_device(jax.devices("cpu")[0]):
        t_lin = np.asarray(jnp.linspace(0.0, TWO_PI, T)).astype(np.float64)
    tfc = (t_lin[:, None] * freq_bands.astype(np.float64)[None, :]
           ).astype(np.float32)                       # = f32(t*f), as jax does
    c_hi = 6.28125                                    # 9-bit-exact split of 2pi
    c_lo = 2.0 * np.pi - c_hi
    k0 = np.round(tfc.astype(np.float64) / (2.0 * np.pi))
    red_hi = (-(k0 * c_hi)).astype(np.float32)        # exact in f32
    red_lo = (-(k0 * c_lo)).astype(np.float32)
    tf = np.ascontiguousarray(
        np.stack([tfc.reshape(-1), red_hi.reshape(-1),
                  red_lo.reshape(-1)]).astype(np.float32))
    ident = np.eye(128, dtype=np.float32)

    in_maps = []
    for c in range(NCORES):
        t0, t1 = c * TOK, (c + 1) * TOK
        embT = emb_f[t0:t1].T
        if has_bias:
            embT = np.vstack([embT, np.ones((1, TOK), np.float32),
                              np.zeros((127, TOK), np.float32)])
        in_maps.append({
            "emb": emb_f[t0:t1],
            "embT": np.ascontiguousarray(embT.astype(np.float32)),
            "noise": noise_f[t0:t1],
            "rr": np.ascontiguousarray(rr_f[t0:t1].transpose(1, 0, 2)),
            "pr": np.ascontiguousarray(pr_f[t0:t1].transpose(1, 0, 2)),
            "W": W2,
            "tf": tf,
            "ident": ident,
        })
    return in_maps, (w0, w1, w2, w3), has_bias


_cache = {}


def kernel(embeddings, pop_W, pop_b, freq_bands, enc_weights,
           rate_noise, rate_rand, pop_rand, _want_trace=False):
    in_maps, (w0, w1, w2, w3), has_bias = _prepare_inputs(
        embeddings, pop_W, pop_b, freq_bands, enc_weights,
        rate_noise, rate_rand, pop_rand)

    key = (w0, w1, w2, w3, has_bias)
    if key not in _cache:
        _cache[key] = _build_program(w0, w1, w2, w3, has_bias)
    nc = _cache[key]

    res = run_bass_kernel_spmd(nc, in_maps, core_ids=list(range(NCORES)),
                               trace=_want_trace)

    # out per core: [T, TOK, D] -> full [B, T, S, D]
    full = np.empty((NTOK, T, D), np.float32)
    for c in range(NCORES):
        full[c * TOK:(c + 1) * TOK] = res.results[c]["out"].transpose(1, 0, 2)
    out = full.reshape(B, S, T, D).transpose(0, 2, 1, 3)
    out = np.ascontiguousarray(out)
    if _want_trace:
        kernel._last_trace = res
    return out



# revision 7
# speedup vs baseline: 1.6559x; 1.6559x over previous
"""Trainium2 Bass kernel for the BreakthroughSNN encoder problem.

Computation (per (b, t, s, d) element, w = softmax(enc_weights)):
    rates   = clip(sigmoid(emb)*0.9 + 0.05 + 0.1*noise, 0, 1)          [b,s,d]
    rate    = 1[rate_rand < rates]                                     [b,t,s,d]
    st      = floor(sigmoid(emb) * (T-1))                              [b,s,d]
    temporal= 1[st == t]                                               [b,t,s,d]
    presp   = emb @ pop_W + pop_b ; prates = sigmoid(presp)            [b,s,d,n]
    pop     = mean_n 1[pop_rand < prates]                              [b,t,s,d]
    waves   = sin(freq_d * t_k + sigmoid(emb)*2pi)                     [b,t,s,d]
    phase   = 1[waves > 0.5]                                           [b,t,s,d]
    out     = w0*rate + w1*temporal + w2*pop + w3*phase

Design notes (v2, transposed):
  * Sharding: (b, s) token axis (1024 tokens) split over 8 cores, 128/core.
  * On-chip layout is FEATURE-major ("transposed"): partition p = d % 128,
    free = (dc, tok) with dc = d // 128.  This lets the N=8 population-spike
    sum run on the PE as 8 identity-matmul PSUM accumulations (instead of a
    DVE halving tree), and lets the per-(t,d) wave offsets enter via tiny
    K=2 matmuls.
  * rate_rand/pop_rand are pre-quantized on host to uint16 fixed point
    (floor(x*65536)); on-chip thresholds are likewise scaled to u16.  DVE
    compares two u16 tensors at 2x rate; quantization error ~2^-17.
  * pop_W / emb^T for the matmul are bf16 (PE 4x faster than fp32); the
    resulting prates error ~3e-4 is far inside the 2e-2 gate.
  * Waves: arg = phases + tfred accumulated in PSUM from bf16 hi/lo splits
    (exact to ~2e-5).  arg in (-pi, 3pi) is folded into the Sin-valid range
    with s = Sign(arg - pi) (ACT), v = arg - pi*s (DVE), sin(v - pi) (ACT,
    bias folds the shift).
  * Output written bf16: all outputs lie on an exact 1/32 grid.
"""

import os
import sys

for _p in ("/opt/trn_rl_repo", os.path.expanduser("~/.axon_site/_ro/trn_rl_repo")):
    if os.path.isdir(_p) and _p not in sys.path:
        sys.path.insert(0, _p)

import numpy as np
import ml_dtypes

import concourse.bacc as bacc
import concourse.mybir as mybir
import concourse.tile as tile
from concourse.bass_utils import run_bass_kernel_spmd

Alu = mybir.AluOpType
Act = mybir.ActivationFunctionType
F32 = mybir.dt.float32
F16 = mybir.dt.float16
U16 = mybir.dt.uint16
F16_NP = np.float16

TWO_PI = 2.0 * np.pi
PI_F = float(np.float32(np.pi))

B, T, S, D, N = 4, 16, 256, 512, 8
NCORES = 8
NTOK = B * S                 # 1024 tokens
TOK = NTOK // NCORES         # 128 tokens per core
DC = D // 128                # 4 feature chunks
FD = DC * TOK                # 512 = free size of a [128, (dc, tok)] tile
DNF = N * FD                 # 4096 = free size of pop tiles
KC = D // 128                # 4 contraction chunks for the pop matmul


def _build_program(w0, w1, w2, w3, has_bias):
    """Single-core Bass/Tile program (run SPMD on 8 cores)."""
    from contextlib import ExitStack

    uniform = abs(w1 - w0) < 1e-12 and abs(w3 - w0) < 1e-12 and abs(w2 - w0) < 1e-12

    nc = bacc.Bacc("TRN2", target_bir_lowering=False, debug=False,
                   num_devices=NCORES)

    embT32 = nc.dram_tensor("embT32", [128, FD], F32, kind="ExternalInput")
    embT16 = nc.dram_tensor("embT16", [128, FD], F16, kind="ExternalInput")
    noiseT = nc.dram_tensor("noiseT", [128, FD], F32, kind="ExternalInput")
    rrd = nc.dram_tensor("rrd", [T, 128, FD], U16, kind="ExternalInput")
    prd = nc.dram_tensor("prd", [T, 128, DNF], U16, kind="ExternalInput")
    Wd = nc.dram_tensor("Wd", [128, N * DC * KC * 128], F16, kind="ExternalInput")
    tfd = nc.dram_tensor("tfd", [2, T * FD], F16, kind="ExternalInput")
    identd = nc.dram_tensor("identd", [128, 128], F16, kind="ExternalInput")
    bd = nc.dram_tensor("bd", [128, N * DC], F32, kind="ExternalInput")
    outd = nc.dram_tensor("outd", [T, 128, FD], F16, kind="ExternalOutput")

    with tile.TileContext(nc) as tc, ExitStack() as ctx:
        const = ctx.enter_context(tc.tile_pool(name="const", bufs=1))
        wpool = ctx.enter_context(tc.tile_pool(name="wpool", bufs=1))
        psA = ctx.enter_context(tc.tile_pool(name="psA", bufs=2, space="PSUM"))
        psB = ctx.enter_context(tc.tile_pool(name="psB", bufs=2, space="PSUM"))
        psW = ctx.enter_context(tc.tile_pool(name="psW", bufs=2, space="PSUM"))
        lp = ctx.enter_context(tc.tile_pool(name="lp", bufs=2))
        sp = ctx.enter_context(tc.tile_pool(name="sp", bufs=2))

        # ---- one-time loads ----
        ident = const.tile([128, 128], F16)
        nc.sync.dma_start(ident[:], identd[:])
        emb32 = const.tile([128, FD], F32)
        nc.sync.dma_start(emb32[:], embT32[:])
        emb16 = const.tile([128, FD], F16)
        nc.sync.dma_start(emb16[:], embT16[:])
        noise = const.tile([128, FD], F32)
        nc.sync.dma_start(noise[:], noiseT[:])
        tf = const.tile([2, T * FD], F16)
        nc.sync.dma_start(tf[:], tfd[:])
        Wsb = wpool.tile([128, N * DC * KC * 128], F16)
        nc.sync.dma_start(Wsb[:], Wd[:])
        ones2 = const.tile([2, 128], F16)
        nc.vector.memset(ones2[:], 1.0)
        neg_pi = const.tile([128, 1], F32)
        nc.vector.memset(neg_pi[:], -PI_F)
        neg_half = const.tile([128, 1], F32)
        nc.vector.memset(neg_half[:], -0.5)
        if has_bias:
            bsb = const.tile([128, N * DC], F32)
            nc.sync.dma_start(bsb[:], bd[:])

        # ---- per-token precompute (all [128, FD] transposed layout) ----
        sig = const.tile([128, FD], F32)
        nc.scalar.activation(sig[:], emb32[:], Act.Sigmoid)

        # rates_q = u16(clip(sig*0.9+0.05+0.1*noise, 0, 1) * 65536, capped)
        tmp = const.tile([128, FD], F32)
        nc.vector.tensor_scalar(tmp[:], sig[:], 0.9, 0.05, Alu.mult, Alu.add)
        nc.vector.scalar_tensor_tensor(tmp[:], noise[:], 0.1, tmp[:],
                                       Alu.mult, Alu.add)
        tmp2 = const.tile([128, FD], F32)
        nc.vector.tensor_scalar(tmp2[:], tmp[:], 0.0, 65536.0, Alu.max, Alu.mult)
        rates_q = const.tile([128, FD], U16)
        nc.vector.tensor_scalar(rates_q[:], tmp2[:], 65535.0, None, Alu.min)

        # st = floor(sig*15) as bf16 (exact: integers 0..15)
        x15 = const.tile([128, FD], F32)
        nc.vector.tensor_scalar(x15[:], sig[:], float(T - 1), None, Alu.mult)
        rnd = const.tile([128, FD], F32)
        nc.vector.tensor_scalar(rnd[:], x15[:], 8388608.0, 8388608.0,
                                Alu.add, Alu.subtract)
        gtt = const.tile([128, FD], F32)
        nc.vector.tensor_tensor(gtt[:], rnd[:], x15[:], Alu.is_gt)
        st = const.tile([128, FD], F16)
        nc.vector.tensor_tensor(st[:], rnd[:], gtt[:], Alu.subtract)

        # phases = sig*2pi, split into bf16 hi+lo (hi+lo == phases to ~2^-18)
        phases = const.tile([128, FD], F32)
        nc.vector.tensor_scalar(phases[:], sig[:], TWO_PI, None, Alu.mult)
        ph_hi = const.tile([128, FD], F16)
        nc.vector.tensor_scalar(ph_hi[:], phases[:], 0.0, None, Alu.add)
        ph_lo = const.tile([128, FD], F16)
        nc.vector.tensor_tensor(ph_lo[:], phases[:], ph_hi[:], Alu.subtract)

        # ---- pop linear: prates_q[p, g*FD + dc*128 + tok] (u16) ----
        prates_q = const.tile([128, DNF], U16)
        prf = const.tile([128, FD], F32, tag="prf")
        for g in range(N):
            ps = psW.tile([128, FD], F32, tag="pw")
            for dc in range(DC):
                for kc in range(KC):
                    blk = ((g * DC + dc) * KC + kc) * 128
                    nc.tensor.matmul(ps[:, dc * 128:(dc + 1) * 128],
                                     Wsb[:, blk:blk + 128],
                                     emb16[:, kc * 128:(kc + 1) * 128],
                                     start=(kc == 0),
                                     stop=(kc == KC - 1 and not has_bias))
                if has_bias:
                    # bias row enters as a K=1 matmul: b_col.T @ ones
                    nc.tensor.matmul(ps[:, dc * 128:(dc + 1) * 128],
                                     bsb[:, g * DC + dc:g * DC + dc + 1],
                                     ones2[0:1, :],
                                     start=False, stop=True)
            nc.scalar.activation(prf[:], ps[:], Act.Sigmoid)
            nc.vector.tensor_scalar(prates_q[:, g * FD:(g + 1) * FD], prf[:],
                                    65536.0, 65535.0, Alu.mult, Alu.min)

        c_pop = (w2 / w0) / N

        # ---- t-loop ----
        for t in range(T):
            pr_t = lp.tile([128, DNF], U16, tag="pr")
            nc.sync.dma_start(pr_t[:], prd[t])
            rr_t = lp.tile([128, FD], U16, tag="rr")
            nc.sync.dma_start(rr_t[:], rrd[t])

            # pop spikes + PE n-sum (8 identity matmuls accumulate in PSUM)
            spk = sp.tile([128, DNF], F16, tag="spk")
            nc.vector.tensor_tensor(spk[:], pr_t[:], prates_q[:], Alu.is_lt)
            ps_pop = psA.tile([128, FD], F32, tag="pp")
            for g in range(N):
                nc.tensor.matmul(ps_pop[:], ident[:],
                                 spk[:, g * FD:(g + 1) * FD],
                                 start=(g == 0), stop=(g == N - 1))

            # waves arg = ph_hi + ph_lo + tf_hi + tf_lo  (PSUM, f32)
            ps_arg = psB.tile([128, FD], F32, tag="pa")
            nc.tensor.matmul(ps_arg[:], ident[:], ph_hi[:],
                             start=True, stop=False)
            nc.tensor.matmul(ps_arg[:], ident[:], ph_lo[:],
                             start=False, stop=False)
            for dc in range(DC):
                sl = slice(t * FD + dc * 128, t * FD + (dc + 1) * 128)
                nc.tensor.matmul(ps_arg[:, dc * 128:(dc + 1) * 128],
                                 tf[:, sl], ones2[:],
                                 start=False, stop=True)

            # fold arg in (-pi, 3pi) to Sin domain:
            #   s = sign(arg - pi); v = arg - pi*s; waves = sin(v - pi)
            s_arg = sp.tile([128, FD], F16, tag="sa")
            nc.scalar.activation(s_arg[:], ps_arg[:], Act.Sign, bias=neg_pi[:])
            v = sp.tile([128, FD], F32, tag="v")
            nc.vector.scalar_tensor_tensor(v[:], s_arg[:], -PI_F, ps_arg[:],
                                           Alu.mult, Alu.add)
            wv = sp.tile([128, FD], F32, tag="wv")
            nc.scalar.activation(wv[:], v[:], Act.Sin, bias=neg_pi[:])
            # s3 = sign(waves - 0.5) in {-1,0,1}; spike = (s3+1)/2
            s3 = sp.tile([128, FD], F16, tag="s3")
            nc.scalar.activation(s3[:], wv[:], Act.Sign, bias=neg_half[:])

            # accumulate: rate + temporalA + temporalB + 0.5*s3 + c_pop*pops
            sA = sp.tile([128, FD], F16, tag="sA")
            sB = sp.tile([128, FD], F16, tag="sB")
            nc.vector.tensor_tensor(sA[:], rr_t[:], rates_q[:], Alu.is_lt)
            if uniform:
                nc.vector.scalar_tensor_tensor(sB[:], st[:], t - 0.5, sA[:],
                                               Alu.is_gt, Alu.add)
                nc.vector.scalar_tensor_tensor(sA[:], st[:], t + 0.5, sB[:],
                                               Alu.is_lt, Alu.add)
                nc.vector.scalar_tensor_tensor(sB[:], s3[:], 0.5, sA[:],
                                               Alu.mult, Alu.add)
                sE = sp.tile([128, FD], F32, tag="sE")
                nc.vector.scalar_tensor_tensor(sE[:], ps_pop[:], c_pop, sB[:],
                                               Alu.mult, Alu.add)
                ot = sp.tile([128, FD], F16, tag="ot")
                # out = w0*(acc - 1 + 0.5) = w0*acc - 0.5*w0
                nc.scalar.activation(ot[:], sE[:], Act.Copy,
                                     bias=-0.5 * w0, scale=w0)
            else:
                r1 = w1 / w0
                r3 = w3 / w0
                tA = sp.tile([128, FD], F16, tag="tA")
                nc.vector.tensor_scalar(tA[:], st[:], t - 0.5, r1,
                                        Alu.is_gt, Alu.mult)
                nc.vector.tensor_tensor(sB[:], tA[:], sA[:], Alu.add)
                nc.vector.tensor_scalar(tA[:], st[:], t + 0.5, r1,
                                        Alu.is_lt, Alu.mult)
                nc.vector.tensor_tensor(sA[:], tA[:], sB[:], Alu.add)
                nc.vector.scalar_tensor_tensor(sB[:], s3[:], 0.5 * r3, sA[:],
                                               Alu.mult, Alu.add)
                sE = sp.tile([128, FD], F32, tag="sE")
                nc.vector.scalar_tensor_tensor(sE[:], ps_pop[:], c_pop, sB[:],
                                               Alu.mult, Alu.add)
                ot = sp.tile([128, FD], F16, tag="ot")
                # out = w0*acc + w0*(-r1 + 0.5*r3)
                nc.scalar.activation(ot[:], sE[:], Act.Copy,
                                     bias=w0 * (-r1 + 0.5 * r3), scale=w0)
            nc.sync.dma_start(outd[t], ot[:])

    nc.compile()
    return nc


def _prepare_inputs(embeddings, pop_W, pop_b, freq_bands, enc_weights,
                    rate_noise, rate_rand, pop_rand):
    """Host-side sharding + layout transforms -> per-core in_maps."""
    e = np.exp(enc_weights.astype(np.float64)
               - enc_weights.astype(np.float64).max())
    w = (e / e.sum()).astype(np.float32)
    w0, w1, w2, w3 = [float(x) for x in w]

    has_bias = bool(np.any(pop_b != 0))

    # emb/noise transposed per core: [p, dc*128 + tok]
    # token = b*S + s ; core = b*2 + s//128 ; tok = s%128
    def to_T(x):  # [B,S,D] f32 -> [NC, 128, FD]
        return np.ascontiguousarray(
            x.reshape(B, 2, TOK, DC, 128).transpose(0, 1, 4, 3, 2)
            .reshape(NCORES, 128, FD).astype(np.float32))

    embT = to_T(np.asarray(embeddings, np.float32))
    noiT = to_T(np.asarray(rate_noise, np.float32))
    embT16 = embT.astype(F16_NP)

    # rate_rand [B,T,S,D] -> u16 [NC, T, 128, FD]
    r = np.minimum(np.asarray(rate_rand, np.float32) * 65536.0, 65535.0)
    r = r.astype(np.uint16)
    r = np.ascontiguousarray(
        r.reshape(B, T, 2, TOK, DC, 128).transpose(0, 2, 1, 5, 4, 3)
        .reshape(NCORES, T, 128, FD))

    # pop_rand [B,T,S,D,N] -> u16 [NC, T, 128, N*FD], free = (n, dc, tok)
    q = np.minimum(np.asarray(pop_rand, np.float32) * 65536.0, 65535.0)
    q = q.astype(np.uint16)
    q = np.ascontiguousarray(
        q.reshape(B, T, 2, TOK, DC, 128, N).transpose(0, 2, 1, 5, 6, 4, 3)
        .reshape(NCORES, T, 128, DNF))

    # pop_W [D, D*N] -> bf16 lhsT blocks [kp, ((g*DC+dc)*KC+kc)*128 + pp]
    Wb = np.asarray(pop_W, np.float32).reshape(KC, 128, DC, 128, N)
    Wb = np.ascontiguousarray(
        Wb.transpose(1, 4, 2, 0, 3).reshape(128, N * DC * KC * 128)
    ).astype(F16_NP)

    # bias [D*N] -> [128, N*DC] col layout: bcol[pp, g*DC+dc] = b[(dc*128+pp)*N+g]
    bvec = np.asarray(pop_b, np.float32).reshape(DC, 128, N)
    bcol = np.ascontiguousarray(bvec.transpose(1, 2, 0).reshape(128, N * DC))

    # tf rows: tfred = f32(t*freq) reduced mod 2pi into [-pi, pi], then
    # split bf16 hi/lo.  Layout [2, t*FD + dc*128 + p].
    import jax
    import jax.numpy as jnp
    with jax.default_device(jax.devices("cpu")[0]):
        t_lin = np.asarray(jnp.linspace(0.0, TWO_PI, T)).astype(np.float32)
    tfc = (t_lin[:, None].astype(np.float32)
           * np.asarray(freq_bands, np.float32)[None, :]).astype(np.float32)
    tfc64 = tfc.astype(np.float64)
    k0 = np.round(tfc64 / (2.0 * np.pi))
    red = tfc64 - (2.0 * np.pi) * k0          # in [-pi, pi], exact f64
    tf_hi = red.astype(F16_NP)
    tf_lo = (red - tf_hi.astype(np.float64)).astype(F16_NP)
    tfrows = np.stack([tf_hi, tf_lo]).astype(F16_NP)     # [2, T, D]
    tfrows = np.ascontiguousarray(tfrows.reshape(2, T * D))  # d = dc*128+p ✓

    ident = np.eye(128, dtype=np.float32).astype(F16_NP)

    in_maps = []
    for c in range(NCORES):
        in_maps.append({
            "embT32": embT[c],
            "embT16": embT16[c],
            "noiseT": noiT[c],
            "rrd": r[c],
            "prd": q[c],
            "Wd": Wb,
            "tfd": tfrows,
            "identd": ident,
            "bd": bcol,
        })
    return in_maps, (w0, w1, w2, w3), has_bias


_cache = {}


def kernel(embeddings, pop_W, pop_b, freq_bands, enc_weights,
           rate_noise, rate_rand, pop_rand, _want_trace=False):
    in_maps, (w0, w1, w2, w3), has_bias = _prepare_inputs(
        embeddings, pop_W, pop_b, freq_bands, enc_weights,
        rate_noise, rate_rand, pop_rand)

    key = (w0, w1, w2, w3, has_bias)
    if key not in _cache:
        _cache[key] = _build_program(w0, w1, w2, w3, has_bias)
    nc = _cache[key]

    res = run_bass_kernel_spmd(nc, in_maps, core_ids=list(range(NCORES)),
                               trace=_want_trace)

    # out per core: [T, 128, FD] bf16, free = (dc, tok) -> full [B, T, S, D]
    full = np.empty((NTOK, T, D), np.float32)
    for c in range(NCORES):
        arr = np.asarray(res.results[c]["outd"]).astype(np.float32)
        arr = arr.reshape(T, 128, DC, TOK).transpose(3, 0, 2, 1)
        full[c * TOK:(c + 1) * TOK] = arr.reshape(TOK, T, D)
    out = full.reshape(B, S, T, D).transpose(0, 2, 1, 3)
    out = np.ascontiguousarray(out)
    if _want_trace:
        kernel._last_trace = res
    return out


# revision 11
# speedup vs baseline: 1.9797x; 1.1955x over previous
"""Trainium2 Bass kernel for the BreakthroughSNN encoder problem.

Computation (per (b, t, s, d) element, w = softmax(enc_weights)):
    rates   = clip(sigmoid(emb)*0.9 + 0.05 + 0.1*noise, 0, 1)          [b,s,d]
    rate    = 1[rate_rand < rates]                                     [b,t,s,d]
    st      = floor(sigmoid(emb) * (T-1))                              [b,s,d]
    temporal= 1[st == t]                                               [b,t,s,d]
    presp   = emb @ pop_W + pop_b ; prates = sigmoid(presp)            [b,s,d,n]
    pop     = mean_n 1[pop_rand < prates]                              [b,t,s,d]
    waves   = sin(freq_d * t_k + sigmoid(emb)*2pi)                     [b,t,s,d]
    phase   = 1[waves > 0.5]                                           [b,t,s,d]
    out     = w0*rate + w1*temporal + w2*pop + w3*phase

Design notes (v3, transposed + PE-push):
  * Sharding: (b, s) token axis (1024 tokens) split over 8 cores, 128/core.
  * On-chip layout is FEATURE-major ("transposed"): partition p = d % 128,
    free = (dc, tok) with dc = d // 128.
  * Logit trick: pop_rand is shipped as f16 logit(pop_rand); the Bernoulli
    test 1[rand < sigmoid(presp)] becomes 1[logit(rand) < presp], so no
    on-chip sigmoid of presp is needed at all (presp is copied PSUM->SBUF
    f16 by the ACT engine).
  * rate_rand is u16 fixed point (floor(x*65536)) compared against a u16
    threshold; 16-bit DVE compares run in 2x mode.
  * The whole weighted combination accumulates in ONE PSUM bank via scaled
    identity matmuls: c_pop*I @ spikes (x8, the N-sum), I @ temporal-chain,
    (0.5*r3)*I @ sign(waves-.5), and the final output is a single ACT Copy
    from PSUM with scale/bias.
  * Waves: arg = phases + tfred built in PSUM from f16 hi/lo splits
    (ident matmuls for phases, one K=8 matmul with a dc-selector rhs for
    the tf rows).  Fold to the Sin-valid domain via s=Sign(arg-pi) (ACT)
    pushed back as (-pi_hi*I - pi_lo*I) @ s, then sin(arg' - pi) on ACT.
  * Output written f16: all outputs lie on an exact 1/32-style grid.
"""

import os
import sys

for _p in ("/opt/trn_rl_repo", os.path.expanduser("~/.axon_site/_ro/trn_rl_repo")):
    if os.path.isdir(_p) and _p not in sys.path:
        sys.path.insert(0, _p)

import numpy as np

import concourse.bacc as bacc
import concourse.mybir as mybir
import concourse.tile as tile
from concourse.bass_utils import run_bass_kernel_spmd

Alu = mybir.AluOpType
Act = mybir.ActivationFunctionType
F32 = mybir.dt.float32
F16 = mybir.dt.float16
U16 = mybir.dt.uint16
F16_NP = np.float16

TWO_PI = 2.0 * np.pi
PI_F = float(np.float32(np.pi))
PI_HI = float(np.float16(np.pi))            # f16-exact high part of pi
PI_LO = float(np.float16(np.pi - PI_HI))    # f16 low part; hi+lo ~ pi to 3e-7

B, T, S, D, N = 4, 16, 256, 512, 8
NCORES = 8
NTOK = B * S                 # 1024 tokens
TOK = NTOK // NCORES         # 128 tokens per core
DC = D // 128                # 4 feature chunks
FD = DC * TOK                # 512 = free size of a [128, (dc, tok)] tile
DNF = N * FD                 # 4096 = free size of pop tiles
KC = D // 128                # 4 contraction chunks for the pop matmul


def _build_program(w0, w1, w2, w3, has_bias):
    """Single-core Bass/Tile program (run SPMD on 8 cores)."""
    from contextlib import ExitStack

    uniform = abs(w1 - w0) < 1e-12 and abs(w3 - w0) < 1e-12
    r1 = w1 / w0
    r3 = w3 / w0
    c_pop = (w2 / w0) / N

    nc = bacc.Bacc("TRN2", target_bir_lowering=False, debug=False,
                   num_devices=NCORES)

    embT32 = nc.dram_tensor("embT32", [128, FD], F32, kind="ExternalInput")
    embT16 = nc.dram_tensor("embT16", [128, FD], F16, kind="ExternalInput")
    noiseT = nc.dram_tensor("noiseT", [128, FD], F32, kind="ExternalInput")
    rrd = nc.dram_tensor("rrd", [T, 128, FD], U16, kind="ExternalInput")
    prd = nc.dram_tensor("prd", [T, 128, DNF], F16, kind="ExternalInput")
    Wd = nc.dram_tensor("Wd", [N, 128, DC * KC * 128], F16, kind="ExternalInput")
    tfd = nc.dram_tensor("tfd", [8, T * 128], F16, kind="ExternalInput")
    # 5 scaled identities: [I, c_pop*I, (0.5*r3)*I, -pi_hi*I, -pi_lo*I]
    identsd = nc.dram_tensor("identsd", [128, 5 * 128], F16, kind="ExternalInput")
    dcseld = nc.dram_tensor("dcseld", [8, FD], F16, kind="ExternalInput")
    bd = nc.dram_tensor("bd", [N * DC, 128], F32, kind="ExternalInput")
    outd = nc.dram_tensor("outd", [T, 128, FD], F16, kind="ExternalOutput")

    with tile.TileContext(nc) as tc, ExitStack() as ctx:
        const = ctx.enter_context(tc.tile_pool(name="const", bufs=1))
        wpool = ctx.enter_context(tc.tile_pool(name="wpool", bufs=2))
        psA = ctx.enter_context(tc.tile_pool(name="psA", bufs=2, space="PSUM"))
        psB = ctx.enter_context(tc.tile_pool(name="psB", bufs=2, space="PSUM"))
        psW = ctx.enter_context(tc.tile_pool(name="psW", bufs=2, space="PSUM"))
        lp = ctx.enter_context(tc.tile_pool(name="lp", bufs=2))
        sp = ctx.enter_context(tc.tile_pool(name="sp", bufs=2))

        # ---- one-time loads ----
        idents = const.tile([128, 5 * 128], F16)
        nc.sync.dma_start(idents[:], identsd[:])
        ident = idents[:, 0:128]
        identc = idents[:, 128:256]
        identh = idents[:, 256:384]
        identp1 = idents[:, 384:512]
        identp2 = idents[:, 512:640]
        emb32 = const.tile([128, FD], F32)
        nc.sync.dma_start(emb32[:], embT32[:])
        emb16 = const.tile([128, FD], F16)
        nc.sync.dma_start(emb16[:], embT16[:])
        noise = const.tile([128, FD], F32)
        nc.sync.dma_start(noise[:], noiseT[:])
        tf = const.tile([8, T * 128], F16)
        nc.sync.dma_start(tf[:], tfd[:])
        dcsel = const.tile([8, FD], F16)
        nc.sync.dma_start(dcsel[:], dcseld[:])
        neg_pi = const.tile([128, 1], F32)
        nc.vector.memset(neg_pi[:], -PI_F)
        neg_half = const.tile([128, 1], F32)
        nc.vector.memset(neg_half[:], -0.5)
        if has_bias:
            bsb = const.tile([N * DC, 128], F32)
            nc.sync.dma_start(bsb[:], bd[:])
            onesb = const.tile([1, 128], F16)
            nc.vector.memset(onesb[:], 1.0)

        # ---- per-token precompute (all [128, FD] transposed layout) ----
        sig = const.tile([128, FD], F32)
        nc.scalar.activation(sig[:], emb32[:], Act.Sigmoid)

        # rates_q = u16(clip(sig*0.9+0.05+0.1*noise, 0, 1) * 65536, capped)
        tmp = const.tile([128, FD], F32)
        nc.vector.tensor_scalar(tmp[:], sig[:], 0.9, 0.05, Alu.mult, Alu.add)
        nc.vector.scalar_tensor_tensor(tmp[:], noise[:], 0.1, tmp[:],
                                       Alu.mult, Alu.add)
        tmp2 = const.tile([128, FD], F32)
        nc.vector.tensor_scalar(tmp2[:], tmp[:], 0.0, 65536.0, Alu.max, Alu.mult)
        rates_q = const.tile([128, FD], U16)
        nc.vector.tensor_scalar(rates_q[:], tmp2[:], 65535.0, None, Alu.min)

        # st = floor(sig*15) as f16 (exact: integers 0..15)
        x15 = const.tile([128, FD], F32)
        nc.vector.tensor_scalar(x15[:], sig[:], float(T - 1), None, Alu.mult)
        rnd = const.tile([128, FD], F32)
        nc.vector.tensor_scalar(rnd[:], x15[:], 8388608.0, 8388608.0,
                                Alu.add, Alu.subtract)
        gtt = const.tile([128, FD], F32)
        nc.vector.tensor_tensor(gtt[:], rnd[:], x15[:], Alu.is_gt)
        st = const.tile([128, FD], F16)
        nc.vector.tensor_tensor(st[:], rnd[:], gtt[:], Alu.subtract)

        # phases = sig*2pi, split into f16 hi+lo (hi+lo == phases to ~2^-22)
        phases = const.tile([128, FD], F32)
        nc.vector.tensor_scalar(phases[:], sig[:], TWO_PI, None, Alu.mult)
        ph_hi = const.tile([128, FD], F16)
        nc.vector.tensor_scalar(ph_hi[:], phases[:], 0.0, None, Alu.add)
        ph_lo = const.tile([128, FD], F16)
        nc.vector.tensor_tensor(ph_lo[:], phases[:], ph_hi[:], Alu.subtract)

        # ---- pop linear: presp16[p, g*FD + dc*128 + tok] (f16) ----
        # presp = emb @ pop_W (+b); compare is in logit domain so no sigmoid.
        presp16 = const.tile([128, DNF], F16)
        for g in range(N):
            Wg = wpool.tile([128, DC * KC * 128], F16, tag="wg")
            nc.sync.dma_start(Wg[:], Wd[g])
            ps = psW.tile([128, FD], F32, tag="pw")
            for dc in range(DC):
                for kc in range(KC):
                    nc.tensor.matmul(ps[:, dc * 128:(dc + 1) * 128],
                                     Wg[:, (dc * KC + kc) * 128:
                                        (dc * KC + kc + 1) * 128],
                                     emb16[:, kc * 128:(kc + 1) * 128],
                                     start=(kc == 0),
                                     stop=(kc == KC - 1 and not has_bias))
                if has_bias:
                    nc.tensor.matmul(ps[:, dc * 128:(dc + 1) * 128],
                                     bsb[g * DC + dc:g * DC + dc + 1, :],
                                     onesb[0:1, :],
                                     start=False, stop=True)
            nc.scalar.activation(presp16[:, g * FD:(g + 1) * FD], ps[:],
                                 Act.Copy, bias=0.0, scale=1.0)

        # ---- t-loop ----
        for t in range(T):
            pr_t = lp.tile([128, DNF], F16, tag="pr")
            nc.sync.dma_start(pr_t[:], prd[t])
            rr_t = lp.tile([128, FD], U16, tag="rr")
            nc.sync.dma_start(rr_t[:], rrd[t])

            # pop spikes: 1[logit(rand) < presp]
            spk = sp.tile([128, DNF], F16, tag="spk")
            nc.vector.tensor_tensor(spk[:], pr_t[:], presp16[:], Alu.is_lt)

            # waves arg = ph_hi + ph_lo + tf_hi + tf_lo  (PSUM, f32)
            ps_arg = psB.tile([128, FD], F32, tag="pa")
            nc.tensor.matmul(ps_arg[:], ident, ph_hi[:],
                             start=True, stop=False)
            nc.tensor.matmul(ps_arg[:], ident, ph_lo[:],
                             start=False, stop=False)
            # one K=8 matmul adds tfred[t, d] via the dc-selector rhs
            nc.tensor.matmul(ps_arg[:], tf[:, t * 128:(t + 1) * 128],
                             dcsel[:], start=False, stop=True,
                             skip_group_check=True)
            # fold: s = sign(arg - pi); arg += -pi*s (two scaled-ident MMs)
            s_arg = sp.tile([128, FD], F16, tag="sa")
            nc.scalar.activation(s_arg[:], ps_arg[:], Act.Sign, bias=neg_pi[:])
            nc.tensor.matmul(ps_arg[:], identp1, s_arg[:],
                             start=False, stop=False, skip_group_check=True)
            nc.tensor.matmul(ps_arg[:], identp2, s_arg[:],
                             start=False, stop=True, skip_group_check=True)
            wv = sp.tile([128, FD], F32, tag="wv")
            nc.scalar.activation(wv[:], ps_arg[:], Act.Sin, bias=neg_pi[:])
            # s3 = sign(waves - 0.5) in {-1,0,1}; spike = (s3+1)/2
            s3 = sp.tile([128, FD], F16, tag="s3")
            nc.scalar.activation(s3[:], wv[:], Act.Sign, bias=neg_half[:])

            # rate + temporal chain on DVE (f16)
            sA = sp.tile([128, FD], F16, tag="sA")
            sB = sp.tile([128, FD], F16, tag="sB")
            sC = sp.tile([128, FD], F16, tag="sC")
            nc.vector.tensor_tensor(sA[:], rr_t[:], rates_q[:], Alu.is_lt)
            if uniform:
                nc.vector.scalar_tensor_tensor(sB[:], st[:], t - 0.5, sA[:],
                                               Alu.is_gt, Alu.add)
                nc.vector.scalar_tensor_tensor(sC[:], st[:], t + 0.5, sB[:],
                                               Alu.is_lt, Alu.add)
            else:
                tA = sp.tile([128, FD], F16, tag="tA")
                nc.vector.tensor_scalar(tA[:], st[:], t - 0.5, r1,
                                        Alu.is_gt, Alu.mult)
                nc.vector.tensor_tensor(sB[:], tA[:], sA[:], Alu.add)
                nc.vector.tensor_scalar(tA[:], st[:], t + 0.5, r1,
                                        Alu.is_lt, Alu.mult)
                nc.vector.tensor_tensor(sC[:], tA[:], sB[:], Alu.add)

            # everything accumulates in one PSUM bank:
            #   acc = sum_g c_pop*spk_g + sC + (0.5*r3)*s3
            ps_acc = psA.tile([128, FD], F32, tag="pp")
            for g in range(N):
                nc.tensor.matmul(ps_acc[:], identc,
                                 spk[:, g * FD:(g + 1) * FD],
                                 start=(g == 0), stop=False)
            nc.tensor.matmul(ps_acc[:], ident, sC[:],
                             start=False, stop=False)
            nc.tensor.matmul(ps_acc[:], identh, s3[:],
                             start=False, stop=True)

            # out = w0*acc + w0*(-r1 + 0.5*r3)
            ot = sp.tile([128, FD], F16, tag="ot")
            nc.scalar.activation(ot[:], ps_acc[:], Act.Copy,
                                 bias=w0 * (-r1 + 0.5 * r3), scale=w0)
            nc.sync.dma_start(outd[t], ot[:])

    nc.compile()
    return nc


def _prepare_inputs(embeddings, pop_W, pop_b, freq_bands, enc_weights,
                    rate_noise, rate_rand, pop_rand):
    """Host-side sharding + layout transforms -> per-core in_maps."""
    e = np.exp(enc_weights.astype(np.float64)
               - enc_weights.astype(np.float64).max())
    w = (e / e.sum()).astype(np.float32)
    w0, w1, w2, w3 = [float(x) for x in w]

    has_bias = bool(np.any(pop_b != 0))

    # emb/noise transposed per core: [p, dc*128 + tok]
    # token = b*S + s ; core = b*2 + s//128 ; tok = s%128
    def to_T(x):  # [B,S,D] f32 -> [NC, 128, FD]
        return np.ascontiguousarray(
            x.reshape(B, 2, TOK, DC, 128).transpose(0, 1, 4, 3, 2)
            .reshape(NCORES, 128, FD).astype(np.float32))

    embT = to_T(np.asarray(embeddings, np.float32))
    noiT = to_T(np.asarray(rate_noise, np.float32))
    embT16 = embT.astype(F16_NP)

    # rate_rand [B,T,S,D] -> u16 [NC, T, 128, FD]
    r = np.minimum(np.asarray(rate_rand, np.float32) * 65536.0, 65535.0)
    r = r.astype(np.uint16)
    r = np.ascontiguousarray(
        r.reshape(B, T, 2, TOK, DC, 128).transpose(0, 2, 1, 5, 4, 3)
        .reshape(NCORES, T, 128, FD))

    # pop_rand [B,T,S,D,N] -> f16 logit [NC, T, 128, N*FD], free = (n, dc, tok)
    q = np.asarray(pop_rand, np.float32)
    with np.errstate(divide="ignore"):
        q = np.log(q) - np.log1p(-q)        # logit; rand=0 -> -inf (ok)
    q = q.astype(F16_NP)
    q = np.ascontiguousarray(
        q.reshape(B, T, 2, TOK, DC, 128, N).transpose(0, 2, 1, 5, 6, 4, 3)
        .reshape(NCORES, T, 128, DNF))

    # pop_W [D, D*N] -> f16 lhsT blocks Wd[g][kp, (dc*KC+kc)*128 + pp]
    Wb = np.asarray(pop_W, np.float32).reshape(KC, 128, DC, 128, N)
    Wb = np.ascontiguousarray(
        Wb.transpose(4, 1, 2, 0, 3).reshape(N, 128, DC * KC * 128)
    ).astype(F16_NP)

    # bias [D*N] -> [128, N*DC]: bcol[pp, g*DC+dc] = b[(dc*128+pp)*N+g]
    bvec = np.asarray(pop_b, np.float32).reshape(DC, 128, N)
    bcol = np.ascontiguousarray(bvec.transpose(2, 0, 1).reshape(N * DC, 128))

    # tf rows: tfred = f32(t*freq) reduced mod 2pi into [-pi, pi], f16 hi/lo.
    # Layout [dc*2+h, t*128 + pp].
    import jax
    import jax.numpy as jnp
    with jax.default_device(jax.devices("cpu")[0]):
        t_lin = np.asarray(jnp.linspace(0.0, TWO_PI, T)).astype(np.float32)
    tfc = (t_lin[:, None] * np.asarray(freq_bands, np.float32)[None, :]
           ).astype(np.float32)
    tfc64 = tfc.astype(np.float64)
    k0 = np.round(tfc64 / (2.0 * np.pi))
    red = tfc64 - (2.0 * np.pi) * k0          # [T, D] in [-pi, pi], f64
    tf_hi = red.astype(F16_NP)
    tf_lo = (red - tf_hi.astype(np.float64)).astype(F16_NP)
    # [2, T, DC, 128] -> [DC, 2, T, 128] -> [8, T*128]
    tfs = np.stack([tf_hi, tf_lo]).reshape(2, T, DC, 128)
    tfrows = np.ascontiguousarray(
        tfs.transpose(2, 0, 1, 3).reshape(8, T * 128)).astype(F16_NP)

    # dc selector rhs: dcsel[dc'*2+h, dc*128+tok] = 1[dc'==dc]
    dcsel = np.zeros((8, FD), np.float16)
    for dcp in range(DC):
        for h in range(2):
            dcsel[dcp * 2 + h, dcp * 128:(dcp + 1) * 128] = 1.0

    ey = np.eye(128, dtype=np.float32)
    c_pop = (w2 / w0) / N
    r3 = w3 / w0
    idents = np.concatenate(
        [ey, c_pop * ey, (0.5 * r3) * ey, -PI_HI * ey, -PI_LO * ey],
        axis=1).astype(F16_NP)

    in_maps = []
    for c in range(NCORES):
        in_maps.append({
            "embT32": embT[c],
            "embT16": embT16[c],
            "noiseT": noiT[c],
            "rrd": r[c],
            "prd": q[c],
            "Wd": Wb,
            "tfd": tfrows,
            "identsd": idents,
            "dcseld": dcsel,
            "bd": bcol,
        })
    return in_maps, (w0, w1, w2, w3), has_bias


_cache = {}


def kernel(embeddings, pop_W, pop_b, freq_bands, enc_weights,
           rate_noise, rate_rand, pop_rand, _want_trace=False):
    in_maps, (w0, w1, w2, w3), has_bias = _prepare_inputs(
        embeddings, pop_W, pop_b, freq_bands, enc_weights,
        rate_noise, rate_rand, pop_rand)

    key = (w0, w1, w2, w3, has_bias)
    if key not in _cache:
        _cache[key] = _build_program(w0, w1, w2, w3, has_bias)
    nc = _cache[key]

    res = run_bass_kernel_spmd(nc, in_maps, core_ids=list(range(NCORES)),
                               trace=_want_trace)

    # out per core: [T, 128, FD] f16, free = (dc, tok) -> full [B, T, S, D]
    full = np.empty((NTOK, T, D), np.float32)
    for c in range(NCORES):
        arr = np.asarray(res.results[c]["outd"]).astype(np.float32)
        arr = arr.reshape(T, 128, DC, TOK).transpose(3, 0, 2, 1)
        full[c * TOK:(c + 1) * TOK] = arr.reshape(TOK, T, D)
    out = full.reshape(B, S, T, D).transpose(0, 2, 1, 3)
    out = np.ascontiguousarray(out)
    if _want_trace:
        kernel._last_trace = res
    return out
